# revision 12
# baseline (speedup 1.0000x reference)
"""BitLinear (ternary-quantized linear) Trainium2 kernel.

out = (x @ ternary_quantize(W).T) * mean(|W|),  alpha = 0.7

Sharding: tensor-parallel over out_features (8192 -> 8 x 1024). Every core
gets the full activation x (pre-transposed on host to [K, T] layout so all
device DMAs are contiguous) plus its own weight shard, also pre-transposed
to [K, O_shard]. weight_scale = mean(|W|) is computed on-device from the
local shard and completed with a tiny [128,1] AllReduce across the 8 cores.

Device kernel per core (SPMD, identical program, different data):
  phase 0: DMA wT shard (fp32), |w| row-sums (DVE), AllReduce, partition
           all-reduce -> global mean / threshold replicated on 128 partitions
  phase 1: ternary-quantize the shard into bf16 {-1, 0, +1} (exact in bf16)
  phase 2: tiled matmul: lhsT = x[k,t] bf16 (cast in-flight by SWDGE DMA),
           rhs = wq[k,o], accumulate over k in PSUM, scale by mean(|W|) on
           the scalar engine during PSUM->SBUF copy, DMA out.
"""

import numpy as np

import concourse.bass as bass
import concourse.mybir as mybir
import concourse.tile as tile
from concourse import bacc, bass_isa
from concourse.bass_utils import run_bass_kernel_spmd

N_CORES = 8
B, S, IN_F, OUT_F = 4, 2048, 2048, 8192
T_FULL = B * S              # 8192 tokens
K = IN_F                    # contraction dim
OS = OUT_F // N_CORES       # 1024 out-features per core
P = 128
KT = K // P                 # 16 k-tiles
ALPHA = 0.7
N_TOTAL = float(OUT_F * K)  # 2**24, so 1/N_TOTAL is exact in fp32

C = 512                     # token chunk per x DMA
NF = 512                    # matmul moving free dim (one PSUM bank fp32)

LAST_RESULTS = None         # test harness peeks at exec_time_ns here

import os as _os
SKIP = set(filter(None, _os.environ.get("KERNEL_SKIP", "").split(",")))


def _build_program(t_tokens=T_FULL, loops=1, n_cores=N_CORES):
    F32 = mybir.dt.float32
    BF16 = mybir.dt.bfloat16
    AX = mybir.AxisListType.X
    Alu = mybir.AluOpType

    assert t_tokens % C == 0 and C % P == 0 and OS % NF == 0

    nc = bacc.Bacc(
        "TRN2", target_bir_lowering=False, debug=False, num_devices=n_cores
    )
    xT = nc.dram_tensor("xT", [K, t_tokens], F32, kind="ExternalInput").ap()
    wT = nc.dram_tensor("wT", [K, OS], F32, kind="ExternalInput").ap()
    out = nc.dram_tensor("out", [t_tokens, OS], F32, kind="ExternalOutput").ap()

    with tile.TileContext(nc) as tc:
        for _loop in range(loops):
            _build_body(tc, nc, xT, wT, out, t_tokens, n_cores)

    nc.compile()
    return nc


def _build_body(tc, nc, xT, wT, out, t_tokens, n_cores):
    F32 = mybir.dt.float32
    BF16 = mybir.dt.bfloat16
    AX = mybir.AxisListType.X
    Alu = mybir.AluOpType
    if True:
        with (  # noqa: SIM117

            tc.tile_pool(name="wpool", bufs=1) as wpool,
            tc.tile_pool(name="cpool", bufs=1) as cpool,
            tc.tile_pool(name="dram", bufs=1, space="DRAM") as dram,
            tc.tile_pool(name="xpool", bufs=2) as xpool,
            tc.tile_pool(name="opool", bufs=3) as opool,
            tc.tile_pool(name="psum", bufs=3, space="PSUM") as psum_pool,
        ):
            # ---- phase 0: weight shard load + global mean(|W|) ----
            wf = wpool.tile([P, KT, OS], F32)
            nc.sync.dma_start(wf[:], wT.rearrange("(n p) o -> p n o", p=P))

            asum = cpool.tile([P, KT], F32)
            for k in range(KT):
                nc.vector.tensor_reduce(
                    asum[:, k : k + 1], wf[:, k, :], axis=AX, op=Alu.add,
                    apply_absolute_value=True,
                )
            asum1 = cpool.tile([P, 1], F32)
            nc.vector.tensor_reduce(asum1[:], asum[:], axis=AX, op=Alu.add)

            if n_cores > 1:
                cc_in = dram.tile([P, 1], F32)
                cc_out = dram.tile([P, 1], F32)
                nc.sync.dma_start(cc_in[:], asum1[:])
                nc.gpsimd.collective_compute(
                    "AllReduce", Alu.add,
                    replica_groups=[list(range(n_cores))],
                    ins=[cc_in.opt()], outs=[cc_out.opt()],
                )
                gsum = cpool.tile([P, 1], F32)
                nc.sync.dma_start(gsum[:], cc_out[:])
            else:
                gsum = asum1  # single-core (TimelineSim) variant

            tot = cpool.tile([P, 1], F32)
            nc.gpsimd.partition_all_reduce(
                tot[:], gsum[:], channels=P, reduce_op=bass_isa.ReduceOp.add
            )
            # mean = tot * 2**-24 (exact); thr = 0.7 * mean; both replicated
            mean_t = cpool.tile([P, 1], F32)
            nc.vector.tensor_scalar_mul(mean_t[:], tot[:], 1.0 / N_TOTAL)
            thr_t = cpool.tile([P, 1], F32)
            nc.vector.tensor_scalar_mul(thr_t[:], mean_t[:], ALPHA)
            nthr_t = cpool.tile([P, 1], F32)
            nc.vector.tensor_scalar_mul(nthr_t[:], thr_t[:], -1.0)

            # ---- phase 1: ternary quantize -> wq in bf16 (exact values) ----
            wq = wpool.tile([P, KT, OS], BF16)
            for k in range(KT):
                neg = wpool.tile([P, OS], BF16, tag="negtmp")
                # neg = (w <= -thr) in {0,1}
                nc.vector.tensor_scalar(
                    neg[:], wf[:, k, :], nthr_t[:], None, op0=Alu.is_le
                )
                # wq = (w >= thr) - neg  in {-1, 0, 1}
                nc.vector.scalar_tensor_tensor(
                    wq[:, k, :], wf[:, k, :], thr_t[:], neg[:],
                    op0=Alu.is_ge, op1=Alu.subtract,
                )

            # ---- phase 2: matmul sweep over tokens ----
            xT_t = xT.rearrange("(n p) t -> p n t", p=P)
            n_chunks = t_tokens // C
            for tch in range(n_chunks):
                xb = xpool.tile([P, KT, C], BF16)
                # SWDGE DMA with in-flight fp32 -> bf16 cast
                nc.gpsimd.dma_start(
                    xb[:], xT_t[:, :, tch * C : (tch + 1) * C]
                )
                for tsub in range(C // P):
                    t0 = tch * C + tsub * P
                    pos = [
                        psum_pool.tile([P, NF], F32, name=f"po{i}", tag=f"po{i}")
                        for i in range(OS // NF)
                    ]
                    if "mm" not in SKIP:
                        for k in range(KT):
                            lhsT = xb[:, k, tsub * P : (tsub + 1) * P]
                            for oc in range(OS // NF):
                                nc.tensor.matmul(
                                    pos[oc][:],
                                    lhsT,
                                    wq[:, k, oc * NF : (oc + 1) * NF],
                                    start=(k == 0),
                                    stop=(k == KT - 1),
                                )
                    else:
                        for oc in range(OS // NF):
                            nc.vector.memset(pos[oc][:], 0.0)
                    osb = opool.tile([P, OS], F32)
                    if "scale" not in SKIP:
                        for oc in range(OS // NF):
                            # out = psum * mean(|W|), on the scalar engine
                            nc.scalar.mul(
                                osb[:, oc * NF : (oc + 1) * NF], pos[oc][:],
                                mean_t[:],
                            )
                    else:
                        for oc in range(OS // NF):
                            nc.vector.tensor_copy(
                                osb[:, oc * NF : (oc + 1) * NF], pos[oc][:]
                            )
                    if "outdma" not in SKIP:
                        nc.sync.dma_start(out[t0 : t0 + P, :], osb[:])


def kernel(x, weight):
    global LAST_RESULTS
    x = np.asarray(x, dtype=np.float32)
    weight = np.asarray(weight, dtype=np.float32)
    assert x.shape == (B, S, IN_F), x.shape
    assert weight.shape == (OUT_F, IN_F), weight.shape

    xT = np.ascontiguousarray(x.reshape(T_FULL, K).T)
    in_maps = []
    for c in range(N_CORES):
        wTc = np.ascontiguousarray(weight[c * OS : (c + 1) * OS, :].T)
        in_maps.append({"xT": xT, "wT": wTc})

    nc = _build_program()
    res = run_bass_kernel_spmd(nc, in_maps, list(range(N_CORES)))
    LAST_RESULTS = res
    outs = [res.results[c]["out"] for c in range(N_CORES)]
    return np.concatenate(outs, axis=1).reshape(B, S, OUT_F)
